# revision 4
# baseline (speedup 1.0000x reference)
"""Trainium2 Bass kernel for a GNN BasicBlock (sparse conv x2 + BN + residual).

Computes, for full inputs (N=50000 points, C=128 channels, K=27 offsets):
    out = relu(bn2(conv(relu(bn1(conv(x, w1))), w2)) + x)
where conv(x, w)[n] = sum_k x[nbr_idx[n, k]] @ w[k].

Strategy (8 NeuronCores):
  - Shard points (rows of x / nbr_idx) evenly across the 8 cores; replicate
    the feature table and weights.  BN is folded on the host (scale into the
    weights, shift into a per-channel bias applied by the ReLU activation).
  - One Bass program is compiled once and executed twice (layer 1 with
    res=0, layer 2 with res=x); the host reassembles the full feature table
    between executions (the "all-gather").
  - Neighbor gather: GpSimd dma_gather in TRANSPOSE layout with single fp16
    rows (256B elements, half the bytes of the 512B pair trick).  The int16
    index limit (sign-extended by the desc-gen ucode) is beaten by placing
    the in_ap base at row 32768: idx' = row - 32768 in [-32768, 17231], and
    the ucode's unsigned 32-bit multiply-accumulate wraps negative offsets
    back below the base (verified on HW in both layouts).  The ucode trims
    TRAILING negative indices per call, so the host redirects the final
    element of every gather call to an alias row >= 50000 (positive idx')
    holding a copy of the referenced row; alias slots are per (core, call)
    so the table content is identical across cores.
  - Transpose layout lands each gathered row channel-major: element i of a
    call sits at [partition=channel, column i].  The host orders elements as
    i = (kk*mt + h)*128 + q so the npts columns for offset kk are contiguous
    and in point order -> the gather output feeds matmul directly.  No
    parity select, no PE transposes, no PSUM->SBUF bounce copies.
  - Each macro tile (mt=4 point tiles, npts=512) accumulates 27 fp16
    matmuls [128x128]x[128x512] into one PSUM bank; the residual is added
    with an identity matmul (identity DMAed from DRAM) and a single ScalarE
    activation applies relu(acc + bias) on the way out, f16, transposed
    ([C, pts]); the host transposes back and converts to fp32.
  - All gathers ride ONE SWDGE queue: each DMA engine pairs its tx stream
    with its rx (xbar spray) stream in ring order, and concurrent transpose
    calls on different queues interleave per-engine streams, mispairing tx
    data with rx targets (observed on HW).  Each macro still issues 4 calls
    (at slot-group granularity) so matmuls start as soon as a group lands.
    idx loads ride the ACT engine's HWDGE so the gather's engine-tick wait
    only covers them; one tiny warmup gather pays the ~19us SWDGE ring-init
    stall during the constant-load window.
"""

import dataclasses
import sys

if "/opt/trn_rl_repo" not in sys.path:
    sys.path.insert(0, "/opt/trn_rl_repo")

from contextlib import ExitStack

import numpy as np

import concourse.bass as bass
import concourse.tile as tile
from concourse import bacc, mybir
from concourse.bass_utils import run_bass_kernel_spmd

F32 = mybir.dt.float32
F16 = mybir.dt.float16
I16 = mybir.dt.int16

N, C, K = 50000, 128, 27
EPS = 1e-5
NCORES = 8
SHARD = N // NCORES          # 6250 points per core
PT = 128                     # points per tile
NT = -(-SHARD // PT)         # 49 point tiles per core
PTS_PAD = NT * PT            # 6272 padded points per core
MACRO_TILES = 4              # point tiles per macro tile (matmul N = 512)
NQ = 4                       # SWDGE queues used for the gather
BASE = 32768                 # gather in_ap base row (signed-idx mid-base)

_SIZES = []
_t = 0
while _t < NT:
    _SIZES.append(min(MACRO_TILES, NT - _t))
    _t += MACRO_TILES
N_MACROS = len(_SIZES)
CALLS_PER_CORE = N_MACROS * NQ           # gather calls per core per layer
ALIAS0 = N                               # first alias row
NROWS = -(-(N + NCORES * CALLS_PER_CORE) // 16) * 16   # padded table rows
SLOTS_MAX = MACRO_TILES * K
NI_MAX = SLOTS_MAX * PT


def _queue_bounds(slots):
    return [round(i * slots / NQ) for i in range(NQ + 1)]


def build_program(num_devices=NCORES):
    nc = bacc.Bacc(
        "TRN2",
        target_bir_lowering=False,
        debug=False,
        enable_asserts=False,
        num_devices=num_devices,
        num_swdge_queues=NQ,
    )
    x_dram = nc.dram_tensor("x_rows", [NROWS, C], F16, kind="ExternalInput").ap()
    idx_dram = nc.dram_tensor("idx16", [N_MACROS, PT, NI_MAX // 16], I16,
                              kind="ExternalInput").ap()
    w_dram = nc.dram_tensor("w", [C, K * C], F16, kind="ExternalInput").ap()
    id_dram = nc.dram_tensor("ident", [PT, PT], F16, kind="ExternalInput").ap()
    b_dram = nc.dram_tensor("bias", [C, 1], F32, kind="ExternalInput").ap()
    res_dram = nc.dram_tensor("resT", [C, PTS_PAD], F16, kind="ExternalInput").ap()
    out_dram = nc.dram_tensor("outT", [C, PTS_PAD], F16, kind="ExternalOutput").ap()

    # gather source: base at row BASE so sign-extended int16 indices reach
    # rows [0, NROWS) as idx' = row - BASE
    table = dataclasses.replace(
        x_dram[BASE:, :], ap=[[C, NROWS - BASE], [1, C]]
    )

    with tile.TileContext(nc) as tc, ExitStack() as ctx:
        const_pool = ctx.enter_context(tc.tile_pool(name="const", bufs=1))
        idx_pool = ctx.enter_context(tc.tile_pool(name="idx", bufs=4))
        g_pool = ctx.enter_context(tc.tile_pool(name="g", bufs=3))
        res_pool = ctx.enter_context(tc.tile_pool(name="res", bufs=2))
        out_pool = ctx.enter_context(tc.tile_pool(name="out", bufs=3))
        acc_pool = ctx.enter_context(tc.tile_pool(name="acc", bufs=2, space="PSUM"))

        ident = const_pool.tile([PT, PT], F16)
        nc.sync.dma_start(ident[:], id_dram[:, :])
        w_sb = const_pool.tile([C, K * C], F16)
        nc.sync.dma_start(w_sb[:], w_dram[:, :])
        bias_sb = const_pool.tile([C, 1], F32)
        nc.sync.dma_start(bias_sb[:], b_dram[:, :])

        # one tiny warmup gather pays the SWDGE ring-init stall (~19us)
        # during the constant-load window
        wu_idx = const_pool.tile([PT, 8], I16)
        nc.vector.memset(wu_idx[:], 0)
        wu_g = const_pool.tile([PT, 1, PT], F16)
        nc.gpsimd.dma_gather(
            out_ap=wu_g[:, :, :],
            in_ap=table,
            idxs_ap=wu_idx[:, :],
            num_idxs=PT,
            num_idxs_reg=PT,
            elem_size=C,
            elem_step=C,
            transpose=True,
            single_packet=False,
            queue_num=0,
        )

        def emit_macro(m, mt, t0):
            npts = mt * PT
            slots = mt * K
            ni = slots * PT
            # idx rides the ACT engine's HWDGE: the gather's coarse
            # engine-tick wait then only covers this load, not the Sync
            # queue's const/res/out prefetch backlog
            it = idx_pool.tile([PT, ni // 16], I16, tag="idx")
            nc.scalar.dma_start(it[:, :], idx_dram[m][:, : ni // 16])

            g = g_pool.tile([PT, 1, ni], F16, tag="g")
            bounds = _queue_bounds(slots)
            for h in range(NQ):
                s0, s1 = bounds[h], bounds[h + 1]
                if s1 <= s0:
                    continue
                nih = (s1 - s0) * PT
                nc.gpsimd.dma_gather(
                    out_ap=g[:, :, s0 * PT : s1 * PT],
                    in_ap=table,
                    idxs_ap=it[:, s0 * PT // 16 : s1 * PT // 16],
                    num_idxs=nih,
                    num_idxs_reg=nih,
                    elem_size=C,
                    elem_step=C,
                    transpose=True,
                    single_packet=False,
                    # all transpose gathers MUST share one queue: each DMA
                    # engine pairs its tx stream with its rx (xbar spray)
                    # stream in ring order, and concurrent transpose calls
                    # on different queues interleave per-engine streams,
                    # mispairing tx data with rx targets (observed on HW).
                    queue_num=0,
                )

            res_t = res_pool.tile([C, npts], F16)
            nc.sync.dma_start(res_t[:], res_dram[:, t0 * PT : t0 * PT + npts])

            acc = acc_pool.tile([PT, npts], F32, space="PSUM")
            for kk in range(K):
                nc.tensor.matmul(
                    acc[:],
                    lhsT=w_sb[:, kk * C : (kk + 1) * C],
                    rhs=g[:, 0, kk * npts : (kk + 1) * npts],
                    start=(kk == 0),
                    stop=False,
                )
            nc.tensor.matmul(
                acc[:], lhsT=ident[:], rhs=res_t[:], start=False, stop=True
            )
            out_t = out_pool.tile([C, npts], F16)
            nc.scalar.activation(
                out_t[:],
                acc[:],
                mybir.ActivationFunctionType.Relu,
                bias=bias_sb[:, :1],
                scale=1.0,
            )
            nc.sync.dma_start(out_dram[:, t0 * PT : t0 * PT + npts], out_t[:])

        t0 = 0
        for m, mt in enumerate(_SIZES):
            emit_macro(m, mt, t0)
            t0 += mt
    nc.compile()
    return nc


_PROGRAM = None


def _get_program():
    global _PROGRAM
    if _PROGRAM is None:
        _PROGRAM = build_program()
    return _PROGRAM


def _fold_bn(w, g, b, m, v):
    s = (g / np.sqrt(v + EPS)).astype(np.float32)
    t = (b - m * s).astype(np.float32)
    wf = (w * s[None, None, :]).transpose(1, 0, 2).reshape(C, K * C)
    return np.ascontiguousarray(wf, np.float16), t.reshape(C, 1).astype(np.float32)


def prep_indices(nbr_idx, core):
    """Per-shard gather indices (int16, mid-base biased) + alias row refs.

    Returns idx16 [N_MACROS, 128, NI_MAX/16] int16 (16-wrapped + replicated,
    flat order i = (kk*mt+h)*128 + q, values row-BASE) and alias_src
    [CALLS_PER_CORE] int32: original row for each call's final element,
    which is redirected to alias row ALIAS0 + core*CALLS_PER_CORE + call.
    """
    rows = nbr_idx[core * SHARD : (core + 1) * SHARD]
    if rows.shape[0] < PTS_PAD:
        pad = np.zeros((PTS_PAD - rows.shape[0], K), rows.dtype)
        rows = np.concatenate([rows, pad], axis=0)
    idx16 = np.zeros((N_MACROS, PT, NI_MAX // 16), np.int16)
    alias_src = np.zeros(CALLS_PER_CORE, np.int32)
    call = 0
    t0 = 0
    for m, mt in enumerate(_SIZES):
        npts = mt * PT
        slots = mt * K
        ni = slots * PT
        blk = rows[t0 * PT : t0 * PT + npts]            # [npts, K]
        # flat[(kk*mt+h)*128+q] = blk[h*128+q, kk]
        flat = blk.reshape(mt, PT, K).transpose(2, 0, 1).reshape(ni).astype(np.int32)
        bounds = _queue_bounds(slots)
        for h in range(NQ):
            s0, s1 = bounds[h], bounds[h + 1]
            if s1 <= s0:
                continue
            last = s1 * PT - 1
            alias_src[call] = flat[last]
            flat[last] = ALIAS0 + core * CALLS_PER_CORE + call
            call += 1
        idxp = (flat - BASE).astype(np.int16)
        wrapped = idxp.reshape(ni // 16, 16).T          # [16, ni/16]
        idx16[m, :, : ni // 16] = np.tile(wrapped, (PT // 16, 1))
        t0 += mt
    return idx16, alias_src


TRACE = False
LAST_EXEC_NS = []
_EYE = np.eye(128, dtype=np.float16)


def _build_table(feat16, alias_srcs):
    """feat16: [N, C] f16 features; alias_srcs: per-core alias row refs."""
    xt = np.zeros((NROWS, C), np.float16)
    xt[:N] = feat16
    for core in range(NCORES):
        a0 = ALIAS0 + core * CALLS_PER_CORE
        xt[a0 : a0 + CALLS_PER_CORE] = feat16[alias_srcs[core]]
    return xt


def _run_layer(nc, xt, idx_shards, wf, t, res_shards):
    in_maps = []
    for ci in range(NCORES):
        in_maps.append(
            {
                "x_rows": xt,
                "ident": _EYE,
                "idx16": idx_shards[ci],
                "w": wf,
                "bias": t,
                "resT": res_shards[ci],
            }
        )
    r = run_bass_kernel_spmd(nc, in_maps, core_ids=list(range(NCORES)),
                             trace=TRACE)
    if TRACE:
        LAST_EXEC_NS.append(
            (r.exec_time_ns, r.mean_exec_time_ns, r.instructions_and_trace)
        )
    outs = [r.results[ci]["outT"][:, :SHARD].T for ci in range(NCORES)]
    return np.ascontiguousarray(np.concatenate(outs, axis=0), np.float32)


def kernel(x, w1, g1, b1, m1, v1, w2, g2, b2, m2, v2, nbr_idx):
    x = np.ascontiguousarray(x, np.float32)
    nbr_idx = np.ascontiguousarray(nbr_idx, np.int32)
    w1f, t1 = _fold_bn(np.asarray(w1, np.float32), g1, b1, m1, v1)
    w2f, t2 = _fold_bn(np.asarray(w2, np.float32), g2, b2, m2, v2)

    nc = _get_program()
    idx_shards, alias_srcs = [], []
    for ci in range(NCORES):
        i16, al = prep_indices(nbr_idx, ci)
        idx_shards.append(i16)
        alias_srcs.append(al)

    x16 = x.astype(np.float16)
    zero_res = np.zeros((C, PTS_PAD), np.float16)
    out1 = _run_layer(nc, _build_table(x16, alias_srcs), idx_shards, w1f, t1,
                      [zero_res] * NCORES)

    res_shards = []
    for ci in range(NCORES):
        sh = np.zeros((C, PTS_PAD), np.float16)
        sh[:, :SHARD] = x16[ci * SHARD : (ci + 1) * SHARD].T
        res_shards.append(sh)
    out2 = _run_layer(nc, _build_table(out1.astype(np.float16), alias_srcs),
                      idx_shards, w2f, t2, res_shards)
    return out2
